# revision 10
# baseline (speedup 1.0000x reference)
"""Block-diagonal GRU cell on 8 TRN2 NeuronCores — one block per core.

Math per block n (torch GRUCell):
  gi = x_n @ W_ih[n].T + b_ih[n]        (B, 3*BS)
  gh = h_n @ W_hh[n].T + b_hh[n]
  r = sigmoid(gi_r + gh_r); z = sigmoid(gi_z + gh_z)
  ng = tanh(gi_n + r * gh_n)
  h' = ng + z * (h_n - ng)

On-chip layout (per core): everything transposed on host so the
contraction (feature) dim is the SBUF partition dim and gates land on
PSUM partitions — biases then apply as per-partition ACT/DVE operands.
  A  = [W_ih[n].T ; W_hh[n].T]  -> (1024 feat, 1536 gates), j-major
       slot layout [r_j, z_j, n_j] so group DMAs are 6 KB/partition.
  U  = [x_n.T ; h_n.T]          -> (1024 feat, 1024 batch), batch-chunk
       major in DRAM so per-partition runs are 4-8 KB.
  out = h'.T                    -> (512, 1024), un-transposed on host.
Matmul operands are bf16 (tolerance 2e-2; bf16 keeps rel err ~4e-3) —
fp32r already streams 1 col/cycle so bf16 doesn't speed the PE, but it
halves HBM traffic to 5.25 MB/core (fill done ~14 us before the PE
needs it) and enables FWL so LDWEIGHTS hides under the matmul stream.
r/z gates accumulate x- and h-matmuls into one PSUM bank (8 k-steps);
the n gate keeps i_n / h_n in separate banks. Per output row-block j
the r/z/n matmul groups are interleaved so each combine chain overlaps
the next group's matmuls. All bulk DMA rides the Sync HWDGE ring in
exact consumption order (loads first, output stores behind them — by
store time the loads have drained). PE warm-up matmuls on vector-memset
scratch bridge the runtime prologue + first loads, and a few more are
interleaved at the early data gaps, so the PE never idles and the HAM
activity window never re-throttles to half clock. The final group is
split 256/128/64/64 wide so only a 64-wide combine chain trails the
last matmul.
"""

import os
import sys

import numpy as np

try:
    import concourse.bass as bass
except ImportError:  # fresh grading dir: fall back to the repo checkout
    sys.path.insert(0, "/opt/trn_rl_repo")
    import concourse.bass as bass

import ml_dtypes
import concourse.mybir as mybir
import concourse.tile as tile
from concourse import bacc
from concourse.bass import ts
from concourse.bass_utils import run_bass_kernel_spmd

B = 1024            # batch
NB = 8              # blocks == cores
BS = 512            # hidden block size
G3 = 3 * BS         # gates per block (r, z, n)
KF = 1024           # contraction feats per core: 512 input + 512 hidden
P = 128
KT = KF // P        # 8 k-tiles
GT = G3 // P        # 12 gate column groups: 0-3 r, 4-7 z, 8-11 n
NBC = 2             # batch chunks
BC = B // NBC       # 512 (one PSUM bank of fp32)

F32 = mybir.dt.float32
BF16 = mybir.dt.bfloat16
AFT = mybir.ActivationFunctionType
ALU = mybir.AluOpType

_cache: dict = {}
LAST_RESULTS = None  # BassKernelResults of the most recent run (for test.py)


def _build_nc():
    nc = bacc.Bacc("TRN2", target_bir_lowering=False, debug=False, num_devices=NB)
    # A in j-major slot layout: [j, P, (r|z|n), KT, P]
    a_d = nc.dram_tensor("a", [4, P, 3, KT, P], BF16, kind="ExternalInput").ap()
    u_d = nc.dram_tensor("u", [NBC, P, KT, BC], BF16, kind="ExternalInput").ap()
    brz_d = nc.dram_tensor("brz", [P, 12], F32, kind="ExternalInput").ap()
    bn_d = nc.dram_tensor("bn", [P, 8], F32, kind="ExternalInput").ap()
    o_d = nc.dram_tensor("o", [BS, B], F32, kind="ExternalOutput").ap()

    with tile.TileContext(nc) as tc:
        with (
            tc.tile_pool(name="persist", bufs=1) as persist,
            tc.tile_pool(name="tmp", bufs=3) as tmp,
            tc.tile_pool(name="outp", bufs=6) as outp,
            tc.tile_pool(name="psum", bufs=8, space="PSUM") as psum,
        ):
            # PE warm-up scratch: memset on the (idle) vector engine so the
            # first dummy matmul issues right after the runtime prologue,
            # keeping the HAM activity window busy until real data lands →
            # real matmuls run at 2.4 GHz
            wsb = persist.tile([P, BC], BF16, name="wsb")
            nc.vector.memset(wsb[:], 0.0)
            wps = psum.tile([P, BC], F32, name="wps", tag="ps")

            def warm(n):
                for _ in range(n):
                    nc.tensor.matmul(
                        wps[:], wsb[:, :P], wsb[:], start=True, stop=True
                    )

            warm(7)

            # small bias loads ride the gpsimd SWDGE queue, off the bulk path
            brz_sb = persist.tile([P, 12], F32, name="brz_sb")
            nc.gpsimd.dma_start(brz_sb[:], brz_d[:])
            bn_sb = persist.tile([P, 8], F32, name="bn_sb")
            nc.gpsimd.dma_start(bn_sb[:], bn_d[:])

            # Bulk loads: A stream on the Sync HWDGE ring, U stream on the
            # Activation ring — a single in-flight DMA only reaches ~50% of
            # fabric rate, so the two rings run concurrently to saturate the
            # early fill. Each stream is in exact consumption order.
            U = persist.tile([P, NBC, KT, BC], BF16, name="U")
            A = persist.tile([P, GT * KT, P], BF16, name="A")

            def load_slot(j, t):
                s = 3 * j + t
                nc.sync.dma_start(
                    A[:, s * KT : (s + 1) * KT, :], a_d[j][:, t]
                )

            def load_aj(j):
                nc.sync.dma_start(
                    A[:, 3 * j * KT : 3 * (j + 1) * KT, :],
                    a_d[j].rearrange("p t k q -> p (t k) q"),
                )

            def load_u(bc, k0, k1):
                nc.scalar.dma_start(U[:, bc, k0:k1, :], u_d[bc][:, k0:k1, :])

            load_slot(0, 0)          # r0 weights     (sync ring)
            load_slot(0, 1)          # z0
            load_slot(0, 2)          # n0
            load_aj(1)
            load_aj(2)
            load_aj(3)
            load_u(0, 0, 4)          # act ring, concurrent with the A stream
            load_u(0, 4, 8)
            load_u(1, 0, 8)

            # logical gate group -> A slot: slot 3j=r_j (g=j), 3j+1=z_j
            # (g=4+j), 3j+2=n_j (g=8+j)
            def slot_of(g):
                j, kind = g % 4, g // 4
                return 3 * j + kind

            def lhsT(g, k):
                return A[:, slot_of(g) * KT + k, :]

            # persistent per row-block j: r gate, omz = 1-z, zh = z*h
            r_t = [persist.tile([P, B], F32, name=f"r{j}") for j in range(4)]
            omz = [persist.tile([P, B], F32, name=f"omz{j}") for j in range(4)]
            zh = [persist.tile([P, B], F32, name=f"zh{j}") for j in range(4)]

            def mm_group(g, bc, co, w, k0, k1):
                ps = psum.tile([P, w], F32, name="ps", tag="ps")
                for k in range(k0, k1):
                    nc.tensor.matmul(
                        ps[:],
                        lhsT(g, k),
                        U[:, bc, k, co : co + w],
                        start=(k == k0),
                        stop=(k == k1 - 1),
                    )
                return ps

            def combine(j, bc, co, w, ps_i, ps_h):
                # h' = omz*ng + zh, ng = tanh(i_n + b_in + r*(h_n + b_hn))
                c0 = bc * BC + co
                t = tmp.tile([P, w], F32, name="t", tag="t")
                nc.vector.scalar_tensor_tensor(
                    t[:], ps_h[:, :w], bn_sb[:, 4 + j : 5 + j],
                    r_t[j][:, c0 : c0 + w], ALU.add, ALU.mult,
                )
                t2 = tmp.tile([P, w], F32, name="t2", tag="t2")
                nc.vector.tensor_add(t2[:], t[:], ps_i[:, :w])
                nt = tmp.tile([P, w], F32, name="nt", tag="nt")
                nc.scalar.activation(nt[:], t2[:], AFT.Tanh, bias=bn_sb[:, j : j + 1])
                m = tmp.tile([P, w], F32, name="m", tag="m")
                nc.vector.tensor_mul(m[:], omz[j][:, c0 : c0 + w], nt[:])
                o_t = outp.tile([P, w], F32, name="o_t", tag="o_t")
                nc.vector.tensor_add(o_t[:], m[:], zh[j][:, c0 : c0 + w])
                nc.scalar.dma_start(o_d[ts(j, P), c0 : c0 + w], o_t[:])

            for bc in range(NBC):
                for j in range(4):
                    first = bc == 0 and j == 0
                    if first:
                        # early fill: u(bc0,k4:8) lands ~2.5 us after k0:4,
                        # so run every k0:3 matmul first (r, z, and the
                        # complete n-gate i-part) before touching k4:7 —
                        # the PE never waits on the second U chunk
                        ps_r = psum.tile([P, BC], F32, name="ps", tag="ps")
                        ps_z = psum.tile([P, BC], F32, name="ps", tag="ps")
                        ps_i = psum.tile([P, BC], F32, name="ps", tag="ps")
                        for g, ps in ((0, ps_r), (4, ps_z)):
                            for k in range(4):
                                nc.tensor.matmul(
                                    ps[:], lhsT(g, k), U[:, 0, k, :],
                                    start=(k == 0), stop=False,
                                    skip_group_check=True,
                                )
                        for k in range(4):
                            nc.tensor.matmul(
                                ps_i[:], lhsT(8, k), U[:, 0, k, :],
                                start=(k == 0), stop=(k == 3),
                                skip_group_check=True,
                            )
                        for g, ps in ((0, ps_r), (4, ps_z)):
                            for k in range(4, KT):
                                nc.tensor.matmul(
                                    ps[:], lhsT(g, k), U[:, 0, k, :],
                                    start=False, stop=(k == KT - 1),
                                    skip_group_check=True,
                                )
                    else:
                        ps_r = mm_group(j, bc, 0, BC, 0, KT)
                    nc.scalar.activation(
                        r_t[j][:, ts(bc, BC)], ps_r[:], AFT.Sigmoid,
                        bias=brz_sb[:, j : j + 1],
                    )
                    if not first:
                        ps_z = mm_group(4 + j, bc, 0, BC, 0, KT)
                    zt = tmp.tile([P, BC], F32, name="zt", tag="zt")
                    nc.scalar.activation(
                        zt[:], ps_z[:], AFT.Sigmoid, bias=brz_sb[:, 4 + j : 5 + j]
                    )
                    # 1 - sigmoid(x) == sigmoid(-x); bias col 8+j holds -b_z
                    nc.scalar.activation(
                        omz[j][:, ts(bc, BC)], ps_z[:], AFT.Sigmoid,
                        bias=brz_sb[:, 8 + j : 9 + j], scale=-1.0,
                    )
                    nc.vector.tensor_mul(
                        zh[j][:, ts(bc, BC)], zt[:], U[:, bc, 4 + j, :]
                    )
                    if bc == NBC - 1 and j == 3:
                        # final group: shrinking n-gate slices so only a
                        # 64-wide combine chain trails the last matmul
                        co = 0
                        for w in (BC // 2, BC // 4, BC // 8, BC // 8):
                            ps_h = mm_group(8 + j, bc, co, w, 4, KT)
                            ps_i = mm_group(8 + j, bc, co, w, 0, 4)
                            combine(j, bc, co, w, ps_i, ps_h)
                            co += w
                    else:
                        ps_h = mm_group(8 + j, bc, 0, BC, 4, KT)
                        if not first:
                            ps_i = mm_group(8 + j, bc, 0, BC, 0, 4)
                        combine(j, bc, 0, BC, ps_i, ps_h)

    nc.compile()
    return nc


_SLOT_TO_G = [g for j in range(4) for g in (j, 4 + j, 8 + j)]


def _prep_core_inputs(x, h, W_ih, W_hh, b_ih, b_hh, n):
    a_full = np.concatenate([W_ih[n].T, W_hh[n].T], axis=0)       # (1024, 1536)
    a_sl = a_full.reshape(KT, P, GT, P).transpose(2, 1, 0, 3)[_SLOT_TO_G]
    a_re = np.ascontiguousarray(
        a_sl.reshape(4, 3, P, KT, P).transpose(0, 2, 1, 3, 4)
    ).astype(ml_dtypes.bfloat16)                                  # (4, P, 3, KT, P)
    ut = np.concatenate(
        [x[:, n * BS : (n + 1) * BS].T, h[:, n * BS : (n + 1) * BS].T], axis=0
    ).astype(ml_dtypes.bfloat16)                                  # (1024 feat, B)
    u = np.ascontiguousarray(
        ut.reshape(KT, P, NBC, BC).transpose(2, 1, 0, 3)
    )                                                             # (NBC, P, KT, BC)
    brz8 = (b_ih[n, : 2 * BS] + b_hh[n, : 2 * BS]).reshape(8, P).T  # (P, 8)
    brz = np.ascontiguousarray(
        np.concatenate([brz8, -brz8[:, 4:8]], axis=1)
    )                                                             # (P, 12)
    bn = np.ascontiguousarray(
        np.concatenate(
            [b_ih[n, 2 * BS :].reshape(4, P).T, b_hh[n, 2 * BS :].reshape(4, P).T],
            axis=1,
        )
    )                                                             # (P, 8)
    return {"a": a_re, "u": u, "brz": brz, "bn": bn}


def kernel(x, h, W_ih, W_hh, b_ih, b_hh):
    global LAST_RESULTS
    x = np.asarray(x, dtype=np.float32)
    h = np.asarray(h, dtype=np.float32)
    W_ih = np.asarray(W_ih, dtype=np.float32)
    W_hh = np.asarray(W_hh, dtype=np.float32)
    b_ih = np.asarray(b_ih, dtype=np.float32)
    b_hh = np.asarray(b_hh, dtype=np.float32)

    if "nc" not in _cache:
        _cache["nc"] = _build_nc()
    nc = _cache["nc"]

    in_maps = [
        _prep_core_inputs(x, h, W_ih, W_hh, b_ih, b_hh, n) for n in range(NB)
    ]
    trace = os.environ.get("BASS_KERNEL_TRACE") == "1"
    res = run_bass_kernel_spmd(nc, in_maps, list(range(NB)), trace=trace)
    LAST_RESULTS = res
    return np.concatenate([res.results[n]["o"].T for n in range(NB)], axis=1)
